# revision 8
# baseline (speedup 1.0000x reference)
"""Trainium2 Bass kernel for a K=1 neighborhood-attention block.

Reference computation (per batch b, N=2048 positions, C=512 channels):
    Q  = x @ Wq^T + bq ;  K = x @ Wk^T + bk ;  V = x @ Wv^T + bv
    s[n]   = Q[n] . K[nbr[n]] + rel_bias[0,0]
    scores = one-hot-sparse [N, N]: row n has s[n] at column nbr[n], zeros else
    probs  = softmax(scores / sqrt(C))
    out    = probs @ V[nbr] ;  y = out @ Wo^T + bo

Because each score row is all-zeros except one entry, softmax collapses
to per-row scalars w0 = 1/(e^t + N-1), w1 = 1 - N*w0 applied to two
dense GEMMs (weight folding A = Wq^T Wk, B = Wv^T Wo^T):
    s[n] = x[n] A xg[n]^T + (bias terms);  XB = x @ B
    y[n] = w1[n] * (XB[nbr2[n]] + beta) + w0[n] * S''

Device does the O(N*C^2) work: XA = x @ A on all N rows and XB' = xD @ B
on only the DISTINCT rows xD = x[unique(nbr2)] (~952 of 2048, padded to
a whole number of 128-row tiles) -- the only XB rows the output ever
reads. Host does the O(N*C) epilogue exactly in f32.

Per-core program (fp8 DoubleRow matmuls, x-stationary, streaming the
folded weights): XA phase = 16 tiles x 2 matmuls, XB phase = NTB tiles
x 2 matmuls. PSUM pool of 4 x [128,2,512] (all 8 banks) keeps the PE 4
pairs ahead of the evacuations, which alternate DVE/ACT per pair.
Inputs are issued critical-piece-first (x tile0, then A kc-pair0) as
fat 128-descriptor DMAs over the sync/scalar/gpsimd rings; outputs ship
in 4-tile pieces from the gpsimd ring as soon as their evacs land, and
the final pair is evacuated split across DVE+ACT and shipped on two
rings to shorten the tail. Data-parallel over batch: 8 batches on 8
cores, weights replicated.
"""

import os

import numpy as np

# Recover wedged NeuronCores from a previous crashed run at NRT init.
os.environ.setdefault("NEURON_RT_RESET_CORES", "1")

B, N, C = 8, 2048, 512
P = 128
NT = N // P          # 16 n-tiles for XA
KC = C // P          # 4 contraction chunks
FD = 512             # matmul moving free dim / psum bank

# main-matmul dtype: float8e4 (DoubleRow, fastest), bfloat16, float32r, float32
MM_DT = os.environ.get("NAB_MM_DT", "float8e4")

_TRACE = {"enabled": False, "trace_cores": None, "last": None}
_CACHE = {}


def _np_dt(name):
    import ml_dtypes

    return {
        "bfloat16": ml_dtypes.bfloat16,
        "float8e4": ml_dtypes.float8_e4m3,
    }.get(name, np.float32)


def _build_program(mm_dt_str, ntb):
    import concourse.tile as tile
    from concourse import bacc, mybir

    mm_dt = getattr(mybir.dt, mm_dt_str)
    f32 = mybir.dt.float32
    dr = mm_dt_str == "float8e4" and os.environ.get("NAB_DR", "1") == "1"
    kstep = 2 if dr else 1
    pmode = mybir.MatmulPerfMode.DoubleRow if dr else None
    nkc = KC // kstep    # matmuls per psum slot

    nc = bacc.Bacc("TRN2", target_bir_lowering=False, debug=False)

    # DRAM I/O, pre-tiled host-side: partition dim first, per-partition
    # data contiguous per tile.
    xt_d = nc.dram_tensor("xt", [P, NT, KC, P], mm_dt, kind="ExternalInput")
    xd_d = nc.dram_tensor("xd", [P, ntb, KC, P], mm_dt, kind="ExternalInput")
    a_d = nc.dram_tensor("a", [P, KC, C], mm_dt, kind="ExternalInput")
    bm_d = nc.dram_tensor("bm", [P, KC, C], mm_dt, kind="ExternalInput")
    za_d = nc.dram_tensor("za", [P, NT, C], mm_dt, kind="ExternalOutput")
    zb_d = nc.dram_tensor("zb", [P, ntb, C], mm_dt, kind="ExternalOutput")

    with tile.TileContext(nc) as tc:
        with (
            tc.tile_pool(name="singles", bufs=1) as singles,
            tc.tile_pool(name="ps", bufs=3, space="PSUM") as ps_pool,
            tc.tile_pool(name="psw", bufs=1, space="PSUM") as psw_pool,
        ):
            xt_sb = singles.tile([P, NT, KC, P], mm_dt)
            xd_sb = singles.tile([P, ntb, KC, P], mm_dt)
            a_sb = singles.tile([P, KC, C], mm_dt)
            bm_sb = singles.tile([P, KC, C], mm_dt)
            za_sb = singles.tile([P, NT, C], mm_dt)
            zb_sb = singles.tile([P, ntb, C], mm_dt)

            xt_ap, xd_ap = xt_d.ap(), xd_d.ap()
            a_ap, bm_ap = a_d.ap(), bm_d.ap()
            za_ap, zb_ap = za_d.ap(), zb_d.ap()

            # Inputs: all five sequencers issue DMAs so every transfer
            # is triggered within ~1.4us and the 16 HW queues stay fed.
            # Queue order (per-queue FIFO follows trigger order) is
            # arranged so arrival always leads the PE's consumption:
            # x tile0 + whole A first, then x tiles in order, bm, xd.
            nh = ntb // 2
            # trigger round 1 (~scope start):
            nc.sync.dma_start(xt_sb[:, 0:1], xt_ap[:, 0:1])      # t0      64KB
            nc.scalar.dma_start(a_sb[:, 0:4], a_ap[:, 0:4])      # A      256KB
            nc.gpsimd.dma_start(xt_sb[:, 1:3], xt_ap[:, 1:3])    # t1-2   128KB
            # trigger round 2 (~+0.7us):
            nc.sync.dma_start(xt_sb[:, 3:5], xt_ap[:, 3:5])      # t3-4   128KB
            nc.scalar.dma_start(xt_sb[:, 5:8], xt_ap[:, 5:8])    # t5-7   192KB
            nc.gpsimd.dma_start(bm_sb[:, 0:4], bm_ap[:, 0:4])    # B      256KB
            # trigger round 3 (~+1.4us):
            nc.sync.dma_start(xt_sb[:, 8:11], xt_ap[:, 8:11])    # t8-10  192KB
            nc.scalar.dma_start(xt_sb[:, 11:13], xt_ap[:, 11:13])  # t11-12
            nc.gpsimd.dma_start(xt_sb[:, 13:16], xt_ap[:, 13:16])  # t13-15
            # trigger round 4 (~+2.1us):
            nc.sync.dma_start(xd_sb[:, 0:nh], xd_ap[:, 0:nh])    # xd lo
            nc.scalar.dma_start(xd_sb[:, nh:ntb], xd_ap[:, nh:ntb])  # xd hi

            # PE warm-up: two tiny matmuls on a memset scratch tile with
            # no input dependencies.  They run as soon as the engines
            # start (~2us before the first input data lands) and absorb
            # the first-matmul pipeline warm-up penalty (~1.5us).
            if dr:
                warm_sb = singles.tile([P, 2, P], mm_dt)
                nc.vector.memset(warm_sb[:], 0)
                psw = psw_pool.tile([P, P], f32, tag="psw")
                for wi in range(2):
                    nc.tensor.matmul(
                        psw[:],
                        warm_sb[:],
                        warm_sb[:],
                        start=(wi == 0),
                        stop=(wi == 1),
                        perf_mode=pmode,
                    )

            def mm_pair(psum, x_tiles, t0, w_sb):
                for j in range(2):
                    for kc in range(0, KC, kstep):
                        nc.tensor.matmul(
                            psum[:, j, :],
                            x_tiles[:, t0 + j, kc : kc + kstep, :],
                            w_sb[:, kc : kc + kstep, :],
                            start=(kc == 0),
                            stop=(kc + kstep == KC),
                            perf_mode=pmode,
                        )

            act_copy = mybir.ActivationFunctionType.Copy

            # XA phase: 16 tiles in 8 pairs; evacs alternate DVE/ACT;
            # ship 4-tile output pieces as soon as both evacs land.
            for pi in range(NT // 2):
                t0 = 2 * pi
                ps = ps_pool.tile([P, 2, FD], f32, tag="ps")
                mm_pair(ps, xt_sb, t0, a_sb)
                if pi % 2 == 0:
                    nc.vector.tensor_copy(za_sb[:, t0 : t0 + 2, :], ps[:])
                else:
                    nc.scalar.activation(
                        out=za_sb[:, t0 : t0 + 2, :], in_=ps[:], func=act_copy
                    )
                    nc.gpsimd.dma_start(
                        za_ap[:, t0 - 2 : t0 + 2], za_sb[:, t0 - 2 : t0 + 2]
                    )

            # XB phase on the packed distinct rows.  Per-tile evacs
            # alternating DVE/ACT with per-tile output pieces on two
            # rings: the final tile's evac starts the moment its psum is
            # done instead of queueing behind a busy engine, keeping the
            # tail at ~evac+DMA of a single 64KB tile.
            for pb in range(nh):
                t0 = 2 * pb
                ps = ps_pool.tile([P, 2, FD], f32, tag="ps")
                mm_pair(ps, xd_sb, t0, bm_sb)
                for j in range(2):
                    t = t0 + j
                    if t % 2 == 0:
                        nc.vector.tensor_copy(zb_sb[:, t, :], ps[:, j, :])
                    else:
                        nc.scalar.activation(
                            out=zb_sb[:, t, :], in_=ps[:, j, :], func=act_copy
                        )
                    [nc.gpsimd, nc.sync][t % 2].dma_start(
                        zb_ap[:, t : t + 1], zb_sb[:, t : t + 1]
                    )

    nc.compile()
    return nc


def kernel(x, neighbors, Wq, bq, Wk, bk, Wv, bv, rel_bias, Wo, bo):
    from concourse.bass_utils import run_bass_kernel_spmd

    x = np.asarray(x, dtype=np.float32)
    Wq = np.asarray(Wq, dtype=np.float32)
    Wk = np.asarray(Wk, dtype=np.float32)
    Wv = np.asarray(Wv, dtype=np.float32)
    Wo = np.asarray(Wo, dtype=np.float32)
    bq = np.asarray(bq, dtype=np.float32)
    bk = np.asarray(bk, dtype=np.float32)
    bv = np.asarray(bv, dtype=np.float32)
    bo = np.asarray(bo, dtype=np.float32)
    rel_bias = np.asarray(rel_bias, dtype=np.float32)
    nbr = np.asarray(neighbors).reshape(N, -1)[:, 0].astype(np.int64)
    nbr2 = nbr[nbr]

    # Only the distinct rows of XB = x @ B are ever read by the output;
    # compute XB' on x[D] padded to whole 128-row tiles.
    D, inv = np.unique(nbr2, return_inverse=True)
    nd = len(D)
    ntb = max(2, -(-nd // (2 * P)) * 2)  # even tile count (pairs), >= 2
    Dpad = np.zeros(ntb * P, dtype=np.int64)
    Dpad[:nd] = D

    mm_np = _np_dt(MM_DT)

    # host-side weight folding (tiny)
    A = (Wq.T @ Wk).astype(np.float32)            # [C, C]
    Bm = (Wv.T @ Wo.T).astype(np.float32)         # [C, C]
    beta = (Wo @ bv + bo).astype(np.float32)      # [C]
    u = (Wq.T @ bk).astype(np.float32)
    v = (Wk.T @ bq).astype(np.float32)
    const = float(bq @ bk) + float(rel_bias[0, 0])

    key = (MM_DT, ntb)
    if key not in _CACHE:
        _CACHE[key] = _build_program(MM_DT, ntb)
    nc = _CACHE[key]

    def tile_T(t, nt):  # [nt*P, C] -> [P, nt, KC, P] (x^T pre-tiled)
        return np.ascontiguousarray(
            t.reshape(nt, P, KC, P).transpose(3, 0, 2, 1)
        )

    A_t = np.ascontiguousarray(A.reshape(KC, P, C).transpose(1, 0, 2)).astype(mm_np)
    Bm_t = np.ascontiguousarray(Bm.reshape(KC, P, C).transpose(1, 0, 2)).astype(mm_np)

    in_maps = [
        {
            "xt": tile_T(x[b], NT).astype(mm_np),
            "xd": tile_T(x[b][Dpad], ntb).astype(mm_np),
            "a": A_t,
            "bm": Bm_t,
        }
        for b in range(B)
    ]

    res = run_bass_kernel_spmd(
        nc,
        in_maps,
        core_ids=list(range(B)),
        trace=_TRACE["enabled"],
        trace_cores=_TRACE["trace_cores"],
    )
    _TRACE["last"] = res

    # host-side O(N*C) epilogue, exact in f32:
    # y[n] = w1[n]*(XB'[inv[n]] + beta) + w0[n]*S'',  S'' = sxg@B + N*beta
    xg = x[:, nbr, :]                             # [B, N, C]
    sbias = x @ u + xg @ v + const                # [B, N]
    S2pp = xg.sum(axis=1) @ Bm + float(N) * beta  # [B, C] = S''

    y = np.empty((B, N, C), dtype=np.float32)
    for b in range(B):
        za = res.results[b]["za"].astype(np.float32)  # [P, NT, C]
        zb = res.results[b]["zb"].astype(np.float32)  # [P, ntb, C]
        XA = np.ascontiguousarray(za.transpose(1, 0, 2)).reshape(N, C)
        XBr = np.ascontiguousarray(zb.transpose(1, 0, 2)).reshape(ntb * P, C)
        s = np.einsum("nc,nc->n", XA, xg[b]) + sbias[b]
        t = s / np.sqrt(C, dtype=np.float32)
        e = np.exp(t)
        w0 = 1.0 / (e + (N - 1))
        w1 = 1.0 - N * w0
        y[b] = w1[:, None] * (XBr[inv] + beta[None, :]) + w0[:, None] * S2pp[b][None, :]
    return y
